# revision 1
# baseline (speedup 1.0000x reference)
# Trainium2 Bass kernel for CrossAttentionFusion — fp8 DoubleRow rewrite.
#
# Reference computation (per batch b):
#   pet_seq = pet_feat[b] viewed as (C, L)^T            L = H*W = 4096, C = 512
#   q = pet_seq @ Wq.T ; k = ct_seq @ Wk.T ; v = ct_seq @ Wv.T   (8 heads, hd=64)
#   x = softmax(q k^T / sqrt(hd)) v                      per head
#   y = LN(pet_seq + x @ Wp.T + bp) * gamma + beta       -> (B, C, H, W)
#
# Sharding: 8 cores = 2 batches x 4 query-row chunks (1024 rows each), no
# collectives.  All heavy matmuls run as fp8e4 (e4m3) with
# perf_mode=DoubleRow (0.5 cyc/output-row, 2 contraction tiles per
# instruction in the cost model):
#   - Q/K/V/out projections: real 2-k-tile DR over 256-channel halves.
#   - O = P^T V: real 2-k-tile DR over 256-key chunks; V carries a ones
#     column so PSUM row 64 accumulates the softmax denominator (M=65).
#   - scores: contraction is only 64 (head dim), so the DR plane dim is a
#     stride-0 broadcast (both planes read the same data -> result is 2x the
#     true product); the q-side inputs are pre-scaled by A8*scale/2 so PSUM
#     comes out as A8*logit directly (A8 = 8/ln2: fp8-Schraudolph slope).
#
# Softmax with a fixed logit shift (softmax is shift invariant; the shift
# keeps exp() outputs well inside fp8 range).  exp runs split across THREE
# engines per m-chunk (one [128,1024] instruction covers a head pair):
#   ACT:      P = fp8(exp(psum/A8 - SH))          (table exp)
#   DVE/GPS:  P = bitcast_fp8(int8((psum + 32) max 0))   (Schraudolph bits;
#             the +32/max-0 makes the int8 bit pattern of e4m3, truncation
#             tuned via B8 = 56.05)
# O-normalization divides by the denominator row directly (AluOp divide);
# the denominator row lives at PSUM/SBUF partition 64, broadcast to 0..63
# with a ones-matmul whose lhsT sits at partition 64 (tile_position rules
# allow base 64 for K<=64).
#
# LayerNorm: channel sums via ones-column f32r matmuls, row stats on [1,512]
# strips, sqrt on ACT, broadcast via K=1 matmuls, divide + gamma/beta apply.
#
# Elementwise work is balanced across ACT/DVE/GPS with a greedy cost model
# (engine clocks 1.2 / 0.96 / 1.2*0.6 GHz, per-instr init overheads).

import numpy as np
import ml_dtypes

import concourse.bacc as bacc
import concourse.bass as bass
import concourse.tile as tile
from concourse import mybir
from concourse import bass_utils
from concourse.alu_op_type import AluOpType
from contextlib import ExitStack

F32 = mybir.dt.float32
F32R = mybir.dt.float32r
BF16 = mybir.dt.bfloat16
FP8 = mybir.dt.float8e4
I8 = mybir.dt.int8
E4 = ml_dtypes.float8_e4m3
DR = mybir.MatmulPerfMode.DoubleRow

B, C, H, W = 2, 512, 64, 64
L = H * W                    # 4096
NH, HD = 8, 64
NCORES = 8
LQ = L // 4                  # 1024 query rows per core
ATT_SCALE = HD ** -0.5       # 1/8
LN_EPS = 1e-5

# fp8 Schraudolph constants (e4m3: 4 exp bits bias 7, 3 mantissa bits).
A8 = 8.0 / np.log(2.0)            # octave slope in bit units
B8 = 56.05                        # 7*8 + truncation/Schraudolph tuning
B8EFF = 32.0                      # bias added before int8 trunc
SH = (B8 - B8EFF) / A8            # effective logit shift (~2.084)
QPRESCALE = float(A8 * ATT_SCALE / 2.0)   # pet8 host prescale (DR doubles)
SC_ACT = float(1.0 / A8)
BIAS_ACT = float(-SH)


def dr0(ap):
    """Stride-0 DoubleRow plane dim: [K, N] -> [K, 2, N] reading data twice."""
    return ap.unsqueeze(1).broadcast_to((ap.shape[0], 2, ap.shape[1]))


class EngineBalancer:
    """Greedy assignment of elementwise ops by modeled cost.

    GPSIMD (Pool) cannot access PSUM (walrus verifier), so any op touching
    PSUM may only go to ACT or DVE; SBUF-only work is steered to GPS.
    """
    COST = {"act": (0.8333, 190.0), "dve": (1.0417, 130.0), "gps": (1.3889, 25.0)}

    def __init__(self):
        self.busy = {"act": 0.0, "dve": 0.0, "gps": 0.0}

    def pick(self, cols, force=None, candidates=("act", "dve")):
        if force is None:
            e = min(candidates,
                    key=lambda k: self.busy[k] + self.COST[k][0] * cols + self.COST[k][1])
        else:
            e = force
        c, i = self.COST[e]
        self.busy[e] += c * cols + i
        return e


def build_nc(debug: bool = False, debug_taps: bool = False):
    nc = bacc.Bacc("TRN2", target_bir_lowering=False, debug=debug,
                   num_devices=NCORES)
    eng = {"act": nc.scalar, "dve": nc.vector, "gps": nc.gpsimd}
    bal = EngineBalancer()

    def e_copy(dst, src, cols, force=None):
        which = bal.pick(cols, force)
        if which == "act":
            nc.scalar.copy(dst, src)
        else:
            eng[which].tensor_copy(dst, src)

    # ---- DRAM I/O ----------------------------------------------------------
    pet8_d = nc.dram_tensor("pet8", [2, 128, 2, LQ], FP8, kind="ExternalInput").ap()
    ct8_d = nc.dram_tensor("ct8", [2, 128, 2, L], FP8, kind="ExternalInput").ap()
    pet16_d = nc.dram_tensor("pet16", [C, LQ], BF16, kind="ExternalInput").ap()
    wq8_d = nc.dram_tensor("wq8", [2, 128, 2, C], FP8, kind="ExternalInput").ap()
    wk8_d = nc.dram_tensor("wk8", [2, 128, 2, C], FP8, kind="ExternalInput").ap()
    wv8_d = nc.dram_tensor("wv8", [2, 128, 2, C], FP8, kind="ExternalInput").ap()
    wp8_d = nc.dram_tensor("wp8", [2, 128, 2, C], FP8, kind="ExternalInput").ap()
    gamma_d = nc.dram_tensor("gamma", [C, 1], F32, kind="ExternalInput").ap()
    beta_d = nc.dram_tensor("beta", [C, 1], F32, kind="ExternalInput").ap()
    bp_d = nc.dram_tensor("bp", [C, 1], F32, kind="ExternalInput").ap()
    out_d = nc.dram_tensor("out", [C, LQ], F32, kind="ExternalOutput").ap()
    taps = {}
    if debug_taps:
        taps["qt"] = nc.dram_tensor("dbg_qt", [C, LQ], FP8, kind="ExternalOutput").ap()
        taps["kt"] = nc.dram_tensor("dbg_kt", [C, L], FP8, kind="ExternalOutput").ap()
        taps["vt"] = nc.dram_tensor("dbg_vt", [16, 128, 2, 528], FP8, kind="ExternalOutput").ap()
        taps["xt"] = nc.dram_tensor("dbg_xt", [4, 128, 2, 512], FP8, kind="ExternalOutput").ap()
        taps["xres"] = nc.dram_tensor("dbg_xres", [C, LQ], F32, kind="ExternalOutput").ap()
        taps["p80"] = nc.dram_tensor("dbg_p80", [128, 2, 1024], I8, kind="ExternalOutput").ap()
        taps["osb0"] = nc.dram_tensor("dbg_osb0", [65, 512], F32, kind="ExternalOutput").ap()

    NC4 = C // 128           # 4 channel chunks of 128
    NM = L // 128            # 32 key m-chunks
    NDM = NM // 2            # 16 double m-chunks
    NLQ = LQ // 512          # 2 query chunks per core

    with tile.TileContext(nc) as tc, ExitStack() as top:
        persist = top.enter_context(tc.tile_pool(name="persist", bufs=1))

        def alloc(shape, dt, tag):
            return persist.tile(shape, dt, tag=tag, name=tag)

        # persistent tensors
        pet16 = [alloc([128, LQ], BF16, f"pet16_{i}") for i in range(NC4)]
        wp8 = [alloc([128, 2, C], FP8, f"wp8_{i}") for i in range(2)]
        gamma = [alloc([128, 1], F32, f"g_{i}") for i in range(NC4)]
        beta = [alloc([128, 1], F32, f"b_{i}") for i in range(NC4)]
        bp = [alloc([128, 1], F32, f"bp_{i}") for i in range(NC4)]

        qt8 = [alloc([128, LQ], FP8, f"qt8_{i}") for i in range(NH // 2)]
        kt8 = [alloc([128, L], FP8, f"kt8_{i}") for i in range(NH // 2)]
        # 66 cols per head: 64 dims + ones (denominator) + zero pad (dual-fp8
        # ldweights requires an even stationary free dim per k-tile)
        v8 = [alloc([128, 2, 528], FP8, f"v8_{i}") for i in range(NDM)]
        xt8 = [[alloc([128, 2, 512], FP8, f"xt8_{lc}_{t}") for t in range(2)]
               for lc in range(NLQ)]
        xres = [alloc([128, LQ], F32R, f"xr_{i}") for i in range(NC4)]

        # constants
        ones_f = alloc([128, 128], F32, "ones_f")
        onesr = alloc([128, 128], F32R, "onesr")
        nc.vector.memset(ones_f[:], 1.0)
        nc.vector.tensor_copy(onesr[:], ones_f[:])
        bias_t = alloc([128, 1], F32, "bias_t")
        nc.vector.memset(bias_t[:], BIAS_ACT)
        eps_t = alloc([128, 1], F32, "eps_t")
        nc.vector.memset(eps_t[:], LN_EPS)

        # ---- phase 1: load + projections -----------------------------------
        with tc.tile_pool(name="ph1", bufs=1) as ph1, \
             tc.tile_pool(name="pj", bufs=2, space="PSUM") as pj:
            def p1load(ap_dram, shape, dt, tag):
                t = ph1.tile(shape, dt, tag=tag, name=tag)
                nc.sync.dma_start(t[:], ap_dram)
                return t
            wq8 = [p1load(wq8_d[t], [128, 2, C], FP8, f"wq8_{t}") for t in range(2)]
            pet8 = [p1load(pet8_d[t], [128, 2, LQ], FP8, f"pet8_{t}") for t in range(2)]
            wk8 = [p1load(wk8_d[t], [128, 2, C], FP8, f"wk8_{t}") for t in range(2)]
            ct8 = [p1load(ct8_d[t], [128, 2, L], FP8, f"ct8_{t}") for t in range(2)]
            wv8 = [p1load(wv8_d[t], [128, 2, C], FP8, f"wv8_{t}") for t in range(2)]
            for t in range(2):
                nc.sync.dma_start(wp8[t][:], wp8_d[t])
            for i in range(NC4):
                nc.sync.dma_start(pet16[i][:], pet16_d[i * 128:(i + 1) * 128, :])
                nc.sync.dma_start(gamma[i][:], gamma_d[i * 128:(i + 1) * 128, :])
                nc.sync.dma_start(beta[i][:], beta_d[i * 128:(i + 1) * 128, :])
                nc.sync.dma_start(bp[i][:], bp_d[i * 128:(i + 1) * 128, :])

            # Q projection: per pair one wide [128,1024] PSUM covering all LQ
            for pair in range(NH // 2):
                ps = pj.tile([128, 1024], F32, tag="pj", name="pj")
                for h in range(2):
                    for t in range(2):
                        nc.tensor.matmul(
                            ps[:, h * 512:(h + 1) * 512],
                            wq8[t][:, :, pair * 128:(pair + 1) * 128],
                            pet8[t][:, :, h * 512:(h + 1) * 512],
                            start=(t == 0), stop=(t == 1), perf_mode=DR)
                e_copy(qt8[pair][:], ps[:], 1024)

            # K projection: per (pair, kcw of 1024 keys)
            for pair in range(NH // 2):
                for kcw in range(L // 1024):
                    ps = pj.tile([128, 1024], F32, tag="pj", name="pj")
                    for h in range(2):
                        sl = slice(kcw * 1024 + h * 512, kcw * 1024 + (h + 1) * 512)
                        for t in range(2):
                            nc.tensor.matmul(
                                ps[:, h * 512:(h + 1) * 512],
                                wk8[t][:, :, pair * 128:(pair + 1) * 128],
                                ct8[t][:, :, sl],
                                start=(t == 0), stop=(t == 1), perf_mode=DR)
                    e_copy(kt8[pair][:, kcw * 1024:(kcw + 1) * 1024], ps[:], 1024)

            # V projection: two m-chunks [128 keys, 512 dims] share a wide PSUM
            for dm in range(NDM):
                ps = pj.tile([128, 1024], F32, tag="pjv", bufs=2, name="pjv")
                for j in range(2):
                    m = 2 * dm + j
                    for t in range(2):
                        nc.tensor.matmul(
                            ps[:, j * 512:(j + 1) * 512],
                            ct8[t][:, :, m * 128:(m + 1) * 128], wv8[t][:],
                            start=(t == 0), stop=(t == 1), perf_mode=DR)
                dst = v8[dm][:].rearrange("p two (h d) -> p two h d", h=NH)[:, :, :, 0:HD]
                src = ps[:].rearrange("p (two h d) -> p two h d", two=2, h=NH)
                e_copy(dst, src, 1024)
            for dm in range(NDM):
                nc.vector.memset(
                    v8[dm][:].rearrange("p two (h d) -> p two h d", h=NH)[:, :, :, HD:HD + 1],
                    1.0)
                nc.vector.memset(
                    v8[dm][:].rearrange("p two (h d) -> p two h d", h=NH)[:, :, :, HD + 1:HD + 2],
                    0.0)

        # ---- phase 2: attention + norm + out-proj + LayerNorm --------------
        with tc.tile_pool(name="osb", bufs=1) as osbp, \
             tc.tile_pool(name="ps_s", bufs=2, space="PSUM") as ps_s, \
             tc.tile_pool(name="ps_o", bufs=1, space="PSUM") as ps_o, \
             tc.tile_pool(name="pt", bufs=1) as ptp, \
             tc.tile_pool(name="pp", bufs=2, space="PSUM") as pp, \
             tc.tile_pool(name="nrm", bufs=2) as nrm, \
             tc.tile_pool(name="tmp", bufs=2) as tmp, \
             tc.tile_pool(name="lrows", bufs=1) as lrows, \
             tc.tile_pool(name="yout", bufs=2) as yout:
            stores = {}          # (pair, lc, slot) -> o_sb tile [65, 512] F32R

            den_tiles = {}

            def attention(pair, lc):
                hA, hB = 2 * pair, 2 * pair + 1
                oA = ps_o.tile([66, 512], F32, tag="oA", name="oA")
                oB = ps_o.tile([66, 512], F32, tag="oB", name="oB")

                def emit_o(dm, p8t):
                    nc.tensor.matmul(oA[:], v8[dm][:, :, hA * 66:hA * 66 + 66],
                                     p8t[:, :, 0:512],
                                     start=(dm == 0), stop=(dm == NDM - 1),
                                     perf_mode=DR)
                    nc.tensor.matmul(oB[:], v8[dm][:, :, hB * 66:hB * 66 + 66],
                                     p8t[:, :, 512:1024],
                                     start=(dm == 0), stop=(dm == NDM - 1),
                                     perf_mode=DR)

                pend = None   # (dm, p8t): O delayed one dm so PE never waits exp
                for dm in range(NDM):
                    p8t = ptp.tile([128, 2, 1024], FP8, tag="p8", bufs=4, name="p8")
                    for j in range(2):
                        m = 2 * dm + j
                        sAB = ps_s.tile([128, 1024], F32, tag="sAB", name="sAB")
                        for h, base in ((0, 0), (1, 64)):
                            nc.tensor.matmul(
                                sAB[:, h * 512:(h + 1) * 512],
                                dr0(kt8[pair][base:base + 64, m * 128:(m + 1) * 128]),
                                dr0(qt8[pair][base:base + 64, lc * 512:(lc + 1) * 512]),
                                perf_mode=DR)
                        which = bal.pick(1024)
                        dst = p8t[:, j, :]
                        if which == "act":
                            nc.scalar.activation(
                                dst, sAB[:], mybir.ActivationFunctionType.Exp,
                                scale=SC_ACT, bias=bias_t[:])
                        else:
                            eng[which].tensor_scalar(
                                dst.bitcast(I8), sAB[:], B8EFF, 0.0,
                                AluOpType.add, AluOpType.max)
                        if pend is not None and j == 1:
                            emit_o(*pend)
                            pend = None
                    if debug_taps and (pair, lc, dm) == (0, 0, 0):
                        nc.sync.dma_start(taps["p80"], p8t[:].bitcast(I8))
                    pend = (dm, p8t)
                emit_o(*pend)
                if lc not in den_tiles:
                    den_tiles[lc] = nrm.tile([NH, 512], F32, tag=f"den{lc}",
                                             name=f"den{lc}")
                for o, slot in ((oA, 0), (oB, 1)):
                    o_sb = osbp.tile([66, 512], F32R,
                                     tag=f"osb_{pair}_{lc}_{slot}",
                                     name=f"osb_{pair}_{lc}_{slot}")
                    e_copy(o_sb[0:65, :], o[0:65, :], 512)
                    stores[(pair, lc, slot)] = o_sb
                    nc.sync.dma_start(den_tiles[lc][2 * pair + slot:2 * pair + slot + 1, :],
                                      o_sb[64:65, :].bitcast(F32))

            def norm_chunk(lc):
                # xt8[ch, q] = O[ch, q] * (1/den[q]).  HW has no tensor-tensor
                # divide: batch the 8 denominator rows via DMA gather, one DVE
                # reciprocal, then scatter f32r rows for the broadcast matmuls.
                ents = [(p, s) for p in range(NH // 2) for s in (0, 1)]
                den = den_tiles[lc]
                bal.pick(512, force="dve")
                nc.vector.reciprocal(den[:], den[:])
                rec = nrm.tile([len(ents), 512], F32R, tag="rec", name=f"rec{lc}")
                bal.pick(512, force="dve")
                nc.vector.tensor_copy(rec[:], den[:])
                for i, (pair, slot) in enumerate(ents):
                    t, j = divmod(pair, 2)
                    o_sb = stores[(pair, lc, slot)]
                    rr = nrm.tile([1, 512], F32R, tag="rr", name="rr")
                    nc.sync.dma_start(rr[:], rec[i:i + 1, :])
                    bden = pp.tile([64, 512], F32, tag="pp", name="bden")
                    nc.tensor.matmul(bden[:], onesr[0:1, 0:64], rr[:])
                    if slot == 0:
                        bal.pick(512, force="dve")
                        nc.vector.tensor_tensor(
                            xt8[lc][t][0:64, j, :], o_sb[0:64, :], bden[:],
                            AluOpType.mult)
                    else:
                        xq = nrm.tile([64, 512], FP8, tag="xq", name="xq")
                        bal.pick(512, force="dve")
                        nc.vector.tensor_tensor(
                            xq[:], o_sb[0:64, :], bden[:], AluOpType.mult)
                        nc.sync.dma_start(xt8[lc][t][64:128, j, :], xq[:])

            def proj_chunk(lc):
                sl = slice(lc * 512, (lc + 1) * 512)
                for it in range(NC4):
                    ps = pp.tile([128, 512], F32, tag="pp", name="psy")
                    for t in range(2):
                        nc.tensor.matmul(ps[:], wp8[t][:, :, it * 128:(it + 1) * 128],
                                         xt8[lc][t][:],
                                         start=(t == 0), stop=(t == 1), perf_mode=DR)
                    # xres = (y + bp) + petT (reads PSUM -> DVE)
                    bal.pick(512, force="dve")
                    nc.vector.scalar_tensor_tensor(
                        xres[it][:, sl], ps[:], bp[it][:], pet16[it][:, sl],
                        AluOpType.add, AluOpType.add)

            stats = {}

            def ln_stats_chunk(lc):
                sl = slice(lc * 512, (lc + 1) * 512)
                psum = pp.tile([1, 512], F32, tag="pp", name="psum_sum")
                for c in range(NC4):
                    nc.tensor.matmul(psum[:], onesr[:, 0:1], xres[c][:, sl],
                                     start=(c == 0), stop=(c == NC4 - 1))
                psq = pp.tile([1, 512], F32, tag="pp", name="psum_sq")
                for c in range(NC4):
                    xsq = tmp.tile([128, 512], F32R, tag="xsq", name="xsq")
                    bal.pick(512, force="gps")
                    nc.gpsimd.tensor_tensor(xsq[:], xres[c][:, sl],
                                            xres[c][:, sl], AluOpType.mult)
                    nc.tensor.matmul(psq[:], onesr[:, 0:1], xsq[:],
                                     start=(c == 0), stop=(c == NC4 - 1))
                mrow = lrows.tile([1, 512], F32R, tag=f"mu{lc}", name=f"mu{lc}")
                m2 = lrows.tile([1, 512], F32, tag=f"m2{lc}", name=f"m2{lc}")
                ve = lrows.tile([1, 512], F32, tag=f"ve{lc}", name=f"ve{lc}")
                stdr = lrows.tile([1, 512], F32R, tag=f"sd{lc}", name=f"sd{lc}")
                bal.pick(512, force="dve")
                nc.vector.tensor_scalar(mrow[:], psum[:], 1.0 / C, None,
                                        AluOpType.mult)
                bal.pick(512, force="gps")
                nc.gpsimd.tensor_tensor(m2[:], mrow[:], mrow[:], AluOpType.mult)
                bal.pick(512, force="dve")
                nc.vector.scalar_tensor_tensor(ve[:], psq[:], 1.0 / C, m2[:],
                                               AluOpType.mult, AluOpType.subtract)
                sdf = lrows.tile([1, 512], F32, tag=f"sf{lc}", name=f"sf{lc}")
                bal.pick(512, force="act")
                nc.scalar.activation(sdf[:], ve[:],
                                     mybir.ActivationFunctionType.Sqrt,
                                     bias=eps_t[0:1, :])
                bal.pick(512, force="dve")
                with nc.allow_low_precision(reason="f32r view of f32 reciprocal"):
                    nc.vector.reciprocal(stdr[:], sdf[:])
                stats[lc] = (mrow, stdr)

            def ln_apply_chunk(lc):
                sl = slice(lc * 512, (lc + 1) * 512)
                mrow, stdr = stats[lc]
                bmu = pp.tile([128, 512], F32, tag="pp", name="bmu")
                bsd = pp.tile([128, 512], F32, tag="pp", name="bsd")
                nc.tensor.matmul(bmu[:], onesr[0:1, :], mrow[:])
                nc.tensor.matmul(bsd[:], onesr[0:1, :], stdr[:])
                # stage broadcasts to SBUF so GPS can run the apply ops
                smu = nrm.tile([128, 512], F32, tag="smu", name="smu")
                ssd = nrm.tile([128, 512], F32, tag="ssd", name="ssd")
                e_copy(smu[:], bmu[:], 512)
                e_copy(ssd[:], bsd[:], 512)
                for c in range(NC4):
                    t = tmp.tile([128, 512], F32, tag="lnt", bufs=2, name="lnt")
                    y = yout.tile([128, 512], F32, tag="y", name="yout")
                    bal.pick(3 * 512, force="gps")
                    nc.gpsimd.tensor_tensor(t[:], xres[c][:, sl], smu[:],
                                            AluOpType.subtract)
                    nc.gpsimd.tensor_tensor(t[:], t[:], ssd[:], AluOpType.mult)
                    nc.gpsimd.tensor_scalar(y[:], t[:], gamma[c][:], beta[c][:],
                                            AluOpType.mult, AluOpType.add)
                    nc.sync.dma_start(out_d[c * 128:(c + 1) * 128, sl], y[:])

            chunks = []
            for lc in range(NLQ):
                for pair in range(NH // 2):
                    attention(pair, lc)
                    if chunks:
                        chunks.pop(0)()
                chunks += [lambda lc=lc: norm_chunk(lc),
                           lambda lc=lc: proj_chunk(lc),
                           lambda lc=lc: ln_stats_chunk(lc),
                           lambda lc=lc: ln_apply_chunk(lc)]
            while chunks:
                chunks.pop(0)()

        if debug_taps:
            for i in range(NH // 2):
                nc.sync.dma_start(taps["qt"][i * 128:(i + 1) * 128, :], qt8[i][:])
                nc.sync.dma_start(taps["kt"][i * 128:(i + 1) * 128, :], kt8[i][:])
            for dm in range(NDM):
                nc.sync.dma_start(taps["vt"][dm], v8[dm][:])
            for lc in range(NLQ):
                for t in range(2):
                    nc.sync.dma_start(taps["xt"][lc * 2 + t], xt8[lc][t][:])
            for i in range(NC4):
                nc.sync.dma_start(taps["xres"][i * 128:(i + 1) * 128, :],
                                  xres[i][:].bitcast(F32))
            nc.sync.dma_start(taps["osb0"], stores[(0, 0, 0)][0:65, :].bitcast(F32))

    nc.compile()
    return nc


def prep_core_inputs(inputs):
    """Shard + lay out the full inputs for the 8 cores."""
    pet = np.asarray(inputs["pet_feat"], np.float32).reshape(B, C, L)
    ct = np.asarray(inputs["ct_feat"], np.float32).reshape(B, C, L)
    bf = ml_dtypes.bfloat16

    def wprep(w):
        # [2(t), 128(p), 2(j), 512(out)]: value = W[out, 256t+128j+p]
        wt = np.ascontiguousarray(np.asarray(w, np.float32).T)  # [in, out]
        return np.ascontiguousarray(
            wt.reshape(2, 2, 128, C).transpose(0, 2, 1, 3)).astype(E4)

    wq8 = wprep(inputs["Wq"])
    wk8 = wprep(inputs["Wk"])
    wv8 = wprep(inputs["Wv"])
    wp8 = wprep(inputs["Wp"])
    gamma = np.asarray(inputs["gamma"], np.float32).reshape(C, 1)
    beta = np.asarray(inputs["beta"], np.float32).reshape(C, 1)
    bp = np.asarray(inputs["bp"], np.float32).reshape(C, 1)

    ct8 = {}
    for b in range(B):
        ct8[b] = np.ascontiguousarray(
            ct[b].reshape(2, 2, 128, L).transpose(0, 2, 1, 3)).astype(E4)

    in_maps = []
    for core in range(NCORES):
        b, jq = divmod(core, 4)
        sl = slice(jq * LQ, (jq + 1) * LQ)
        pet_sl = np.ascontiguousarray(pet[b][:, sl])
        pet8 = np.ascontiguousarray(
            (pet_sl * QPRESCALE).reshape(2, 2, 128, LQ).transpose(0, 2, 1, 3)
        ).astype(E4)
        in_maps.append({
            "pet8": pet8,
            "ct8": ct8[b],
            "pet16": pet_sl.astype(bf),
            "wq8": wq8, "wk8": wk8, "wv8": wv8, "wp8": wp8,
            "gamma": gamma, "beta": beta, "bp": bp,
        })
    return in_maps


def assemble_output(results):
    out = np.empty((B, C, L), np.float32)
    for core in range(NCORES):
        b, jq = divmod(core, 4)
        out[b][:, jq * LQ:(jq + 1) * LQ] = results[core]["out"]
    return out.reshape(B, C, H, W)


_NC_CACHE = {}


def get_nc(debug=False, debug_taps=False):
    key = (debug, debug_taps)
    if key not in _NC_CACHE:
        _NC_CACHE[key] = build_nc(debug=debug, debug_taps=debug_taps)
    return _NC_CACHE[key]


def kernel(**inputs):
    nc = get_nc()
    in_maps = prep_core_inputs(inputs)
    res = bass_utils.run_bass_kernel_spmd(nc, in_maps, list(range(NCORES)))
    return assemble_output(res.results)

